# revision 10
# baseline (speedup 1.0000x reference)
"""NearbyAttention Trainium2 kernel.

Full-input contract: kernel(**inputs) takes the unsharded numpy inputs of
nn_NearbyAttention (q,k,v: [16,1025,1024] f32; Wq/Wk/Wv/Wo: [1024,1024] f32;
bo: [1024] f32) and returns the full [16,1025,1024] f32 output.

Strategy: 8-way data parallel over the batch dim (2 batches per NeuronCore),
weights replicated, no collectives. Host pre-transposes activations to
[dim, tokens] and casts to bf16. On device, per batch:
  - projections on the PE array (bf16, fp32 PSUM accumulation)
  - sparse "nearby" attention exploiting the 5x5 locality mask:
    key chunks of 128 (4 patch rows) x 256-query windows in S^T layout,
    exp on the scalar engine, multiplicative mask, PV accumulation with a
    ones-column producing the softmax denominator for free
  - output projection back to [tokens, 1024] fp32
"""

import numpy as np
import ml_dtypes

import concourse.bass as bass
import concourse.mybir as mybir
import concourse.tile as tile
from concourse import bacc
from concourse.bass_utils import run_bass_kernel_spmd

BF16 = mybir.dt.bfloat16
F32 = mybir.dt.float32
AF = mybir.ActivationFunctionType
ALU = mybir.AluOpType

B = 16              # full batch
BPC = 2             # batches per core
NCORES = 8
NT = 1025           # tokens (BOS + 32*32 grid)
G = 1024            # grid tokens
DIM = 1024
HEADS = 16
DH = 64
INNER = HEADS * DH  # 1024
P = 128
SCALE = DH ** -0.5  # 0.125
NEG = -30.0         # mask bias for the handful of memset'd cells

KC = 8              # key chunks of 128 grid tokens (4 patch rows each)


def _mask_pattern() -> np.ndarray:
    """[128 key-local, 256 query-local] 0/1 bf16 mask in S^T orientation.

    Key chunk rows kr=0..3 (absolute 4kc+kr); query window rows
    rho=0..7 (absolute 4kc-2+rho). Unmasked iff |kr+2-rho|<=2 and
    |kcol-qcol|<=2.
    """
    kr = np.arange(128)[:, None] // 32
    kcol = np.arange(128)[:, None] % 32
    qr = np.arange(256)[None, :] // 32
    qcol = np.arange(256)[None, :] % 32
    m = (np.abs(kr + 2 - qr) <= 2) & (np.abs(kcol - qcol) <= 2)
    return m.astype(ml_dtypes.bfloat16)


def _qwin(kc: int) -> tuple[int, int, int]:
    """Grid-query window for key chunk kc: (grid_start, width, mask_col_off)."""
    if kc == 0:
        return 0, 192, 64
    if kc == KC - 1:
        return 128 * kc - 64, 192, 0
    return 128 * kc - 64, 256, 0


def build_nc():
    nc = bacc.Bacc("TRN2", target_bir_lowering=False, debug=False,
                   num_devices=NCORES)

    qt = nc.dram_tensor("qt", [BPC, DIM, NT], BF16, kind="ExternalInput")
    kt = nc.dram_tensor("kt", [BPC, DIM, NT], BF16, kind="ExternalInput")
    vt = nc.dram_tensor("vt", [BPC, DIM, NT], BF16, kind="ExternalInput")
    wq = nc.dram_tensor("wq", [DIM, INNER], BF16, kind="ExternalInput")
    wk = nc.dram_tensor("wk", [DIM, INNER], BF16, kind="ExternalInput")
    wv = nc.dram_tensor("wv", [DIM, INNER], BF16, kind="ExternalInput")
    wo = nc.dram_tensor("wo", [INNER, DIM], BF16, kind="ExternalInput")
    out = nc.dram_tensor("out", [BPC, NT, DIM], F32, kind="ExternalOutput")

    mask_np = _mask_pattern()  # [128, 256]
    mask2_np = np.concatenate([mask_np, mask_np], axis=1)  # [128, 512]
    mask_dram = nc.inline_tensor(mask2_np, name="mask2")

    with tile.TileContext(nc) as tc:
        with (
            tc.tile_pool(name="singles", bufs=1) as singles,
            tc.tile_pool(name="perbatch", bufs=1) as perbatch,
            tc.tile_pool(name="hppool", bufs=3) as hppool,
            tc.tile_pool(name="ppool", bufs=4) as ppool,
            tc.tile_pool(name="small", bufs=4) as small,
            tc.tile_pool(name="bcast", bufs=2) as bcast,
            tc.tile_pool(name="psA", bufs=3, space="PSUM") as psA,
            tc.tile_pool(name="psPV", bufs=2, space="PSUM") as psPV,
        ):
            # ---- persistent weights/constants ----
            wq_sb = singles.tile([P, 8, INNER], BF16, tag="wq")
            wk_sb = singles.tile([P, 8, INNER], BF16, tag="wk")
            wv_sb = singles.tile([P, 8, INNER], BF16, tag="wv")
            wo_sb = singles.tile([P, 8, DIM], BF16, tag="wo")
            mask2_sb = singles.tile([P, 512], BF16, tag="mask2")
            ones_sb = singles.tile([P, 1], BF16, tag="ones")

            nc.sync.dma_start(wq_sb[:], wq.ap().rearrange("(c p) n -> p c n", p=P))
            nc.sync.dma_start(wk_sb[:], wk.ap().rearrange("(c p) n -> p c n", p=P))
            nc.sync.dma_start(wv_sb[:], wv.ap().rearrange("(c p) n -> p c n", p=P))
            nc.sync.dma_start(wo_sb[:], wo.ap().rearrange("(c p) n -> p c n", p=P))
            nc.sync.dma_start(mask2_sb[:], mask_dram[:])
            nc.vector.memset(ones_sb[:], 1.0)

            for b in range(BPC):
                # ---- load transposed activations ----
                qT = perbatch.tile([P, 8, NT], BF16, tag="qT")
                kT = perbatch.tile([P, 8, NT], BF16, tag="kT")
                vT = perbatch.tile([P, 8, NT], BF16, tag="vT")
                nc.sync.dma_start(qT[:], qt[b].rearrange("(c p) n -> p c n", p=P))
                nc.sync.dma_start(kT[:], kt[b].rearrange("(c p) n -> p c n", p=P))
                nc.sync.dma_start(vT[:], vt[b].rearrange("(c p) n -> p c n", p=P))

                # ---- v projection into head panels ----
                # vh_panel[g, kc, h, 0:64] = (v @ Wv)[token g+1, h*64:...]
                # col 64 = ones (softmax denominator trick)
                vh_panel = perbatch.tile([P, KC, HEADS, DH + 1], BF16, tag="vhp")
                vbos_panel = perbatch.tile([1, HEADS, DH + 1], BF16, tag="vbos")
                nc.vector.memset(vh_panel[:, :, :, DH], 1.0)
                nc.vector.memset(vbos_panel[:, :, DH], 1.0)

                for mt in range(KC):  # grid token chunks
                    for half in range(2):
                        acc = psA.tile([P, 512], F32, tag="ps")
                        for ktile in range(8):
                            nc.tensor.matmul(
                                acc[:],
                                vT[:, ktile, 1 + 128 * mt: 1 + 128 * (mt + 1)],
                                wv_sb[:, ktile, 512 * half: 512 * (half + 1)],
                                start=(ktile == 0), stop=(ktile == 7),
                            )
                        nc.vector.tensor_copy(
                            vh_panel[:, mt, 8 * half: 8 * (half + 1), 0:DH],
                            acc.rearrange("p (h d) -> p h d", d=DH),
                        )
                # BOS token of v
                for half in range(2):
                    acc = psA.tile([P, 512], F32, tag="ps")
                    for ktile in range(8):
                        nc.tensor.matmul(
                            acc[0:1, :], vT[:, ktile, 0:1],
                            wv_sb[:, ktile, 512 * half: 512 * (half + 1)],
                            start=(ktile == 0), stop=(ktile == 7),
                        )
                    nc.vector.tensor_copy(
                        vbos_panel[:, 8 * half: 8 * (half + 1), 0:DH],
                        acc[0:1].rearrange("p (h d) -> p h d", d=DH),
                    )

                attnout = perbatch.tile([P, 8, NT], BF16, tag="attnout")
                zq_sb = small.tile([1, HEADS], F32, tag="zq")
                pq_all = perbatch.tile([P, HEADS, 9], BF16, tag="pqall")

                for hp in range(8):
                    # ---- q/k head-pair projections (inner chunk = hp) ----
                    qhT = hppool.tile([P, NT], BF16, tag="qhT")
                    khT = hppool.tile([P, NT], BF16, tag="khT")
                    for dst, w_sb, src, scl in (
                        (qhT, wq_sb, qT, SCALE), (khT, wk_sb, kT, 1.0),
                    ):
                        for nt0, ntw in ((0, 512), (512, 512), (1024, 1)):
                            acc = psA.tile([P, 512], F32, tag="ps")
                            for ktile in range(8):
                                nc.tensor.matmul(
                                    acc[:, 0:ntw],
                                    w_sb[:, ktile, 128 * hp: 128 * (hp + 1)],
                                    src[:, ktile, nt0: nt0 + ntw],
                                    start=(ktile == 0), stop=(ktile == 7),
                                )
                            if scl == 1.0:
                                nc.vector.tensor_copy(
                                    dst[:, nt0: nt0 + ntw], acc[:, 0:ntw])
                            else:
                                nc.scalar.activation(
                                    dst[:, nt0: nt0 + ntw], acc[:, 0:ntw],
                                    AF.Copy, scale=scl)

                    for hh in range(2):
                        h = 2 * hp + hh
                        hrows = slice(64 * hh, 64 * (hh + 1))

                        # ---- BOS-as-key scores for all grid queries ----
                        pbos = small.tile([1, G], BF16, tag="pbos")
                        for c0 in (0, 512):
                            sb = psA.tile([P, 512], F32, tag="ps")
                            nc.tensor.matmul(
                                sb[0:1, :], khT[hrows, 0:1],
                                qhT[hrows, 1 + c0: 1 + c0 + 512],
                                start=True, stop=True,
                            )
                            nc.scalar.activation(
                                pbos[:, c0: c0 + 512], sb[0:1, :], AF.Exp)

                        # ---- BOS-as-query scores over everything ----
                        sq = psA.tile([P, 512], F32, tag="ps")
                        nc.vector.memset(sq[:, 0:1], NEG)
                        nc.tensor.matmul(sq[0:1, 0:1], khT[hrows, 0:1],
                                         qhT[hrows, 0:1], start=True, stop=True)
                        for c in range(8):
                            nc.tensor.matmul(
                                sq[:, 1 + c: 2 + c],
                                khT[hrows, 1 + 128 * c: 1 + 128 * (c + 1)],
                                qhT[hrows, 0:1], start=True, stop=True,
                            )
                        pq = pq_all[:, h, :]
                        nc.scalar.activation(pq, sq[:, 0:9], AF.Exp)
                        zrow = psA.tile([P, 512], F32, tag="ps")
                        nc.tensor.matmul(zrow[0:1, 0:9], ones_sb[:], pq,
                                         start=True, stop=True)
                        nc.vector.tensor_reduce(
                            zq_sb[:, h: h + 1], zrow[0:1, 0:9],
                            axis=mybir.AxisListType.X, op=ALU.add)

                        # ---- main nearby attention ----
                        pv = psPV.tile([DH + 1, G], F32, tag="pv")
                        # BOS key contribution opens the accumulation groups
                        nc.tensor.matmul(pv[:, 0:512], vbos_panel[:, h, :],
                                         pbos[:, 0:512], start=True, stop=False,
                                         skip_group_check=True)
                        nc.tensor.matmul(pv[:, 512:G], vbos_panel[:, h, :],
                                         pbos[:, 512:G], start=True, stop=False,
                                         skip_group_check=True)

                        for kcp in range(4):  # pairs of key chunks
                            kca, kcb = 2 * kcp, 2 * kcp + 1
                            s = psA.tile([P, 512], F32, tag="ps")
                            p_sb = ppool.tile([P, 512], BF16, tag="p")
                            for j, kc in enumerate((kca, kcb)):
                                g0, w, _ = _qwin(kc)
                                nc.tensor.matmul(
                                    s[:, 256 * j: 256 * j + w],
                                    khT[hrows, 1 + 128 * kc: 1 + 128 * (kc + 1)],
                                    qhT[hrows, 1 + g0: 1 + g0 + w],
                                    start=True, stop=True,
                                )
                                if w < 256:
                                    nc.vector.memset(
                                        s[:, 256 * j + w: 256 * (j + 1)], 0.0)
                            nc.scalar.activation(p_sb[:], s[:], AF.Exp)
                            # mask (edge chunks use shifted slices)
                            ma = _qwin(kca)[2]
                            mb = _qwin(kcb)[2]
                            if ma == 0 and mb == 0:
                                nc.vector.tensor_tensor(
                                    p_sb[:], p_sb[:], mask2_sb[:], ALU.mult)
                            else:
                                wa = _qwin(kca)[1]
                                wb = _qwin(kcb)[1]
                                nc.vector.tensor_tensor(
                                    p_sb[:, 0:wa], p_sb[:, 0:wa],
                                    mask2_sb[:, ma: ma + wa], ALU.mult)
                                nc.vector.tensor_tensor(
                                    p_sb[:, 256: 256 + wb], p_sb[:, 256: 256 + wb],
                                    mask2_sb[:, mb: mb + wb], ALU.mult)
                            # PV accumulation (split at the PSUM bank boundary)
                            for j, kc in enumerate((kca, kcb)):
                                g0, w, _ = _qwin(kc)
                                pieces = []
                                if g0 < 512 and g0 + w > 512:
                                    pieces = [(g0, 512 - g0), (512, g0 + w - 512)]
                                else:
                                    pieces = [(g0, w)]
                                off = 0
                                for pg0, pw in pieces:
                                    stop = (kc == 4 and pg0 + pw == 512) or \
                                           (kc == 7 and pg0 + pw == G)
                                    nc.tensor.matmul(
                                        pv[:, pg0: pg0 + pw],
                                        vh_panel[:, kc, h, :],
                                        p_sb[:, 256 * j + off: 256 * j + off + pw],
                                        start=False, stop=stop,
                                        skip_group_check=True,
                                    )
                                    off += pw

                        # ---- normalize + evacuate ----
                        # (DVE is lane-aligned: odd heads go through a temp
                        # tile + partition-shifting SBUF->SBUF DMA)
                        rz = small.tile([1, G], F32, tag="rz")
                        nc.vector.reciprocal(rz[:], pv[DH: DH + 1, :])
                        rzb = bcast.tile([64, G], F32, tag="rzb")
                        nc.gpsimd.partition_broadcast(rzb[:], rz[:])
                        if hh == 0:
                            nc.vector.tensor_tensor(
                                attnout[hrows, hp, 1:NT], pv[0:DH, :], rzb[:],
                                ALU.mult)
                        else:
                            tmp = bcast.tile([64, G], BF16, tag="tmp1")
                            nc.vector.tensor_tensor(
                                tmp[:], pv[0:DH, :], rzb[:], ALU.mult)
                            nc.sync.dma_start(attnout[hrows, hp, 1:NT], tmp[:])

                # ---- BOS-query epilogue (needs all heads' zq) ----
                rzq = small.tile([1, HEADS], F32, tag="rzq")
                nc.vector.reciprocal(rzq[:], zq_sb[:])
                rzqb = small.tile([P, HEADS], F32, tag="rzqb")
                nc.gpsimd.partition_broadcast(rzqb[:], rzq[:])
                for h in range(HEADS):
                    hp, hh = divmod(h, 2)
                    hrows = slice(64 * hh, 64 * (hh + 1))
                    pq = pq_all[:, h, :]
                    nc.vector.tensor_tensor(
                        pq, pq,
                        rzqb[:, h: h + 1].to_broadcast([P, 9]), ALU.mult)
                    pvq = psA.tile([P, 512], F32, tag="ps")
                    nc.tensor.matmul(pvq[0:DH + 1, 0:1], vbos_panel[:, h, :],
                                     pq[0:1, 0:1], start=True, stop=False,
                                     skip_group_check=True)
                    for c in range(8):
                        nc.tensor.matmul(
                            pvq[0:DH + 1, 0:1], vh_panel[:, c, h, :],
                            pq[:, 1 + c: 2 + c], start=False, stop=(c == 7),
                            skip_group_check=True,
                        )
                    if hh == 0:
                        nc.vector.tensor_copy(attnout[hrows, hp, 0:1],
                                              pvq[0:DH, 0:1])
                    else:
                        tmpb = small.tile([64, 1], BF16, tag="tmpb")
                        nc.vector.tensor_copy(tmpb[:], pvq[0:DH, 0:1])
                        nc.sync.dma_start(attnout[hrows, hp, 0:1], tmpb[:])

                # ---- output projection ----
                for mt in range(9):
                    t0 = 128 * mt
                    tw = 128 if mt < 8 else 1
                    for half in range(2):
                        acc = psA.tile([P, 512], F32, tag="ps")
                        for ct in range(8):
                            nc.tensor.matmul(
                                acc[0:tw, :],
                                attnout[:, ct, t0: t0 + tw],
                                wo_sb[:, ct, 512 * half: 512 * (half + 1)],
                                start=(ct == 0), stop=(ct == 7),
                            )
                        ost = bcast.tile([P, 512], F32, tag="ost")
                        nc.vector.tensor_copy(ost[0:tw, :], acc[0:tw, :])
                        nc.sync.dma_start(
                            out[b, t0: t0 + tw, 512 * half: 512 * (half + 1)],
                            ost[0:tw, :],
                        )

    nc.compile()
    return nc


_NC = None


def _get_nc():
    global _NC
    if _NC is None:
        _NC = build_nc()
    return _NC


def kernel(q, k, v, Wq, Wk, Wv, Wo, bo):
    bf16 = ml_dtypes.bfloat16
    qT = np.ascontiguousarray(np.asarray(q, np.float32).transpose(0, 2, 1)).astype(bf16)
    kT = np.ascontiguousarray(np.asarray(k, np.float32).transpose(0, 2, 1)).astype(bf16)
    vT = np.ascontiguousarray(np.asarray(v, np.float32).transpose(0, 2, 1)).astype(bf16)
    wq16 = np.asarray(Wq, np.float32).astype(bf16)
    wk16 = np.asarray(Wk, np.float32).astype(bf16)
    wv16 = np.asarray(Wv, np.float32).astype(bf16)
    wo16 = np.asarray(Wo, np.float32).astype(bf16)

    nc = _get_nc()
    in_maps = []
    for c in range(NCORES):
        sl = slice(BPC * c, BPC * (c + 1))
        in_maps.append({
            "qt": np.ascontiguousarray(qT[sl]),
            "kt": np.ascontiguousarray(kT[sl]),
            "vt": np.ascontiguousarray(vT[sl]),
            "wq": wq16, "wk": wk16, "wv": wv16, "wo": wo16,
        })
    res = run_bass_kernel_spmd(nc, in_maps, core_ids=list(range(NCORES)))
    out = np.concatenate([r["out"] for r in res.results], axis=0)
    out = out + np.asarray(bo, np.float32)[None, None, :]
    return out.astype(np.float32)


if __name__ == "__main__":
    rng = np.random.default_rng(0)
    ins = {
        "q": rng.standard_normal((B, NT, DIM), np.float32),
        "k": rng.standard_normal((B, NT, DIM), np.float32),
        "v": rng.standard_normal((B, NT, DIM), np.float32),
        "Wq": rng.standard_normal((DIM, INNER), np.float32) * DIM ** -0.5,
        "Wk": rng.standard_normal((DIM, INNER), np.float32) * DIM ** -0.5,
        "Wv": rng.standard_normal((DIM, INNER), np.float32) * DIM ** -0.5,
        "Wo": rng.standard_normal((INNER, DIM), np.float32) * INNER ** -0.5,
        "bo": np.zeros((DIM,), np.float32),
    }
    o = kernel(**ins)
    print(o.shape, o.dtype, np.abs(o).max())


# revision 13
# speedup vs baseline: 1.2741x; 1.2741x over previous
"""NearbyAttention Trainium2 kernel.

Full-input contract: kernel(**inputs) takes the unsharded numpy inputs of
nn_NearbyAttention (q,k,v: [16,1025,1024] f32; Wq/Wk/Wv/Wo: [1024,1024] f32;
bo: [1024] f32) and returns the full [16,1025,1024] f32 output.

Strategy: 8-way data parallel over the batch dim (2 batches per NeuronCore),
weights replicated, no collectives. Host pre-transposes activations to
[dim, tokens] and casts to bf16. On device, per batch:
  - projections on the PE array (bf16, fp32 PSUM accumulation)
  - sparse "nearby" attention exploiting the 5x5 locality mask:
    key chunks of 128 (4 patch rows) x 256-query windows in S^T layout,
    exp on the scalar engine, multiplicative mask, PV accumulation with a
    ones-column producing the softmax denominator for free
  - output projection back to [tokens, 1024] fp32
"""

import numpy as np
import ml_dtypes

import concourse.bass as bass
import concourse.mybir as mybir
import concourse.tile as tile
from concourse import bacc
from concourse.bass_utils import run_bass_kernel_spmd

BF16 = mybir.dt.bfloat16
F32 = mybir.dt.float32
AF = mybir.ActivationFunctionType
ALU = mybir.AluOpType

B = 16              # full batch
BPC = 2             # batches per core
NCORES = 8
NT = 1025           # tokens (BOS + 32*32 grid)
G = 1024            # grid tokens
DIM = 1024
HEADS = 16
DH = 64
INNER = HEADS * DH  # 1024
P = 128
SCALE = DH ** -0.5  # 0.125
NEG = -30.0         # mask bias for the handful of memset'd cells

KC = 8              # key chunks of 128 grid tokens (4 patch rows each)


def _mask_pattern() -> np.ndarray:
    """[128 key-local, 256 query-local] 0/1 bf16 mask in S^T orientation.

    Key chunk rows kr=0..3 (absolute 4kc+kr); query window rows
    rho=0..7 (absolute 4kc-2+rho). Unmasked iff |kr+2-rho|<=2 and
    |kcol-qcol|<=2.
    """
    kr = np.arange(128)[:, None] // 32
    kcol = np.arange(128)[:, None] % 32
    qr = np.arange(256)[None, :] // 32
    qcol = np.arange(256)[None, :] % 32
    m = (np.abs(kr + 2 - qr) <= 2) & (np.abs(kcol - qcol) <= 2)
    return m.astype(ml_dtypes.bfloat16)


def _qwin(kc: int) -> tuple[int, int, int]:
    """Grid-query window for key chunk kc: (grid_start, width, mask_col_off)."""
    if kc == 0:
        return 0, 192, 64
    if kc == KC - 1:
        return 128 * kc - 64, 192, 0
    return 128 * kc - 64, 256, 0


def build_nc():
    nc = bacc.Bacc("TRN2", target_bir_lowering=False, debug=False,
                   num_devices=NCORES)

    qt = nc.dram_tensor("qt", [BPC, DIM, NT], BF16, kind="ExternalInput")
    kt = nc.dram_tensor("kt", [BPC, DIM, NT], BF16, kind="ExternalInput")
    vt = nc.dram_tensor("vt", [BPC, DIM, NT], BF16, kind="ExternalInput")
    wq = nc.dram_tensor("wq", [DIM, INNER], BF16, kind="ExternalInput")
    wk = nc.dram_tensor("wk", [DIM, INNER], BF16, kind="ExternalInput")
    wv = nc.dram_tensor("wv", [DIM, INNER], BF16, kind="ExternalInput")
    wo = nc.dram_tensor("wo", [INNER, DIM], BF16, kind="ExternalInput")
    out = nc.dram_tensor("out", [BPC, NT, DIM], F32, kind="ExternalOutput")

    mask_np = _mask_pattern()  # [128, 256]
    mask2_np = np.concatenate([mask_np, mask_np], axis=1)  # [128, 512]
    mask_dram = nc.inline_tensor(mask2_np, name="mask2")

    with tile.TileContext(nc) as tc:
        with (
            tc.tile_pool(name="singles", bufs=1) as singles,
            tc.tile_pool(name="perbatch", bufs=1) as perbatch,
            tc.tile_pool(name="hppool", bufs=3) as hppool,
            tc.tile_pool(name="ppool", bufs=3) as ppool,
            tc.tile_pool(name="small", bufs=2) as small,
            tc.tile_pool(name="bcast", bufs=2) as bcast,
            tc.tile_pool(name="psA", bufs=3, space="PSUM") as psA,
            tc.tile_pool(name="psPV", bufs=2, space="PSUM") as psPV,
        ):
            # ---- persistent weights/constants ----
            wq_sb = singles.tile([P, 8, INNER], BF16, tag="wq")
            wk_sb = singles.tile([P, 8, INNER], BF16, tag="wk")
            wv_sb = singles.tile([P, 8, INNER], BF16, tag="wv")
            wo_sb = singles.tile([P, 8, DIM], BF16, tag="wo")
            mask2_sb = singles.tile([P, 512], BF16, tag="mask2")
            ones_sb = singles.tile([P, 1], BF16, tag="ones")

            nc.sync.dma_start(wq_sb[:], wq.ap().rearrange("(c p) n -> p c n", p=P))
            nc.sync.dma_start(wk_sb[:], wk.ap().rearrange("(c p) n -> p c n", p=P))
            nc.sync.dma_start(wv_sb[:], wv.ap().rearrange("(c p) n -> p c n", p=P))
            nc.sync.dma_start(wo_sb[:], wo.ap().rearrange("(c p) n -> p c n", p=P))
            nc.sync.dma_start(mask2_sb[:], mask_dram[:])
            nc.vector.memset(ones_sb[:], 1.0)

            for b in range(BPC):
                # ---- load transposed activations ----
                qT = perbatch.tile([P, 8, NT], BF16, tag="qT")
                kT = perbatch.tile([P, 8, NT], BF16, tag="kT")
                vT = perbatch.tile([P, 8, NT], BF16, tag="vT")
                nc.sync.dma_start(qT[:], qt[b].rearrange("(c p) n -> p c n", p=P))
                nc.sync.dma_start(kT[:], kt[b].rearrange("(c p) n -> p c n", p=P))
                nc.sync.dma_start(vT[:], vt[b].rearrange("(c p) n -> p c n", p=P))

                # ---- v projection into head panels ----
                # vh_panel[g, kc, h, 0:64] = (v @ Wv)[token g+1, h*64:...]
                # col 64 = ones (softmax denominator trick)
                vh_panel = perbatch.tile([P, KC, HEADS, DH + 1], BF16, tag="vhp")
                vbos_panel = perbatch.tile([1, HEADS, DH + 1], BF16, tag="vbos")
                nc.vector.memset(vh_panel[:, :, :, DH], 1.0)
                nc.vector.memset(vbos_panel[:, :, DH], 1.0)

                for mt in range(KC):  # grid token chunks
                    for half in range(2):
                        acc = psA.tile([P, 512], F32, tag="ps")
                        for ktile in range(8):
                            nc.tensor.matmul(
                                acc[:],
                                vT[:, ktile, 1 + 128 * mt: 1 + 128 * (mt + 1)],
                                wv_sb[:, ktile, 512 * half: 512 * (half + 1)],
                                start=(ktile == 0), stop=(ktile == 7),
                            )
                        nc.vector.tensor_copy(
                            vh_panel[:, mt, 8 * half: 8 * (half + 1), 0:DH],
                            acc.rearrange("p (h d) -> p h d", d=DH),
                        )
                # BOS token of v
                for half in range(2):
                    acc = psA.tile([P, 512], F32, tag="ps")
                    for ktile in range(8):
                        nc.tensor.matmul(
                            acc[0:1, :], vT[:, ktile, 0:1],
                            wv_sb[:, ktile, 512 * half: 512 * (half + 1)],
                            start=(ktile == 0), stop=(ktile == 7),
                        )
                    nc.vector.tensor_copy(
                        vbos_panel[:, 8 * half: 8 * (half + 1), 0:DH],
                        acc[0:1].rearrange("p (h d) -> p h d", d=DH),
                    )

                attnout = perbatch.tile([P, 8, NT], BF16, tag="attnout")
                zq_sb = small.tile([1, HEADS], F32, tag="zq")
                pq_all = perbatch.tile([P, HEADS, 9], BF16, tag="pqall")

                for hp in range(8):
                    # ---- q/k head-pair projections (inner chunk = hp) ----
                    qhT = hppool.tile([P, NT], BF16, tag="qhT")
                    khT = hppool.tile([P, NT], BF16, tag="khT")
                    for dst, w_sb, src, scl in (
                        (qhT, wq_sb, qT, SCALE), (khT, wk_sb, kT, 1.0),
                    ):
                        for nt0, ntw in ((0, 512), (512, 512), (1024, 1)):
                            acc = psA.tile([P, 512], F32, tag="ps")
                            for ktile in range(8):
                                nc.tensor.matmul(
                                    acc[:, 0:ntw],
                                    w_sb[:, ktile, 128 * hp: 128 * (hp + 1)],
                                    src[:, ktile, nt0: nt0 + ntw],
                                    start=(ktile == 0), stop=(ktile == 7),
                                )
                            if scl == 1.0:
                                nc.vector.tensor_copy(
                                    dst[:, nt0: nt0 + ntw], acc[:, 0:ntw])
                            else:
                                nc.scalar.activation(
                                    dst[:, nt0: nt0 + ntw], acc[:, 0:ntw],
                                    AF.Copy, scale=scl)

                    for hh in range(2):
                        h = 2 * hp + hh
                        hrows = slice(64 * hh, 64 * (hh + 1))

                        # ---- BOS-as-key scores for all grid queries ----
                        pbos = small.tile([1, G], BF16, tag="pbos")
                        for c0 in (0, 512):
                            sb = psA.tile([P, 512], F32, tag="ps")
                            nc.tensor.matmul(
                                sb[0:1, :], khT[hrows, 0:1],
                                qhT[hrows, 1 + c0: 1 + c0 + 512],
                                start=True, stop=True,
                            )
                            nc.scalar.activation(
                                pbos[:, c0: c0 + 512], sb[0:1, :], AF.Exp)

                        # ---- BOS-as-query scores over everything ----
                        sq = psA.tile([P, 512], F32, tag="ps")
                        nc.vector.memset(sq[:, 0:1], NEG)
                        nc.tensor.matmul(sq[0:1, 0:1], khT[hrows, 0:1],
                                         qhT[hrows, 0:1], start=True, stop=True)
                        for c in range(8):
                            nc.tensor.matmul(
                                sq[:, 1 + c: 2 + c],
                                khT[hrows, 1 + 128 * c: 1 + 128 * (c + 1)],
                                qhT[hrows, 0:1], start=True, stop=True,
                            )
                        pq = pq_all[:, h, :]
                        nc.scalar.activation(pq, sq[:, 0:9], AF.Exp)
                        zrow = psA.tile([P, 512], F32, tag="ps")
                        nc.tensor.matmul(zrow[0:1, 0:9], ones_sb[:], pq,
                                         start=True, stop=True)
                        nc.vector.tensor_reduce(
                            zq_sb[:, h: h + 1], zrow[0:1, 0:9],
                            axis=mybir.AxisListType.X, op=ALU.add)

                        # ---- main nearby attention ----
                        pv = psPV.tile([DH + 1, G], F32, tag="pv")
                        # BOS key contribution opens the accumulation groups
                        nc.tensor.matmul(pv[:, 0:512], vbos_panel[:, h, :],
                                         pbos[:, 0:512], start=True, stop=False,
                                         skip_group_check=True)
                        nc.tensor.matmul(pv[:, 512:G], vbos_panel[:, h, :],
                                         pbos[:, 512:G], start=True, stop=False,
                                         skip_group_check=True)

                        for kcp in range(4):  # pairs of key chunks
                            kca, kcb = 2 * kcp, 2 * kcp + 1
                            s = psA.tile([P, 512], F32, tag="ps")
                            p_sb = ppool.tile([P, 512], BF16, tag="p")
                            for j, kc in enumerate((kca, kcb)):
                                g0, w, _ = _qwin(kc)
                                nc.tensor.matmul(
                                    s[:, 256 * j: 256 * j + w],
                                    khT[hrows, 1 + 128 * kc: 1 + 128 * (kc + 1)],
                                    qhT[hrows, 1 + g0: 1 + g0 + w],
                                    start=True, stop=True,
                                )
                                if w < 256:
                                    nc.vector.memset(
                                        s[:, 256 * j + w: 256 * (j + 1)], 0.0)
                            nc.scalar.activation(p_sb[:], s[:], AF.Exp)
                            # mask (edge chunks use shifted slices)
                            ma = _qwin(kca)[2]
                            mb = _qwin(kcb)[2]
                            if ma == 0 and mb == 0:
                                nc.vector.tensor_tensor(
                                    p_sb[:], p_sb[:], mask2_sb[:], ALU.mult)
                            else:
                                wa = _qwin(kca)[1]
                                wb = _qwin(kcb)[1]
                                nc.vector.tensor_tensor(
                                    p_sb[:, 0:wa], p_sb[:, 0:wa],
                                    mask2_sb[:, ma: ma + wa], ALU.mult)
                                nc.vector.tensor_tensor(
                                    p_sb[:, 256: 256 + wb], p_sb[:, 256: 256 + wb],
                                    mask2_sb[:, mb: mb + wb], ALU.mult)
                            # PV accumulation (split at the PSUM bank boundary)
                            for j, kc in enumerate((kca, kcb)):
                                g0, w, _ = _qwin(kc)
                                pieces = []
                                if g0 < 512 and g0 + w > 512:
                                    pieces = [(g0, 512 - g0), (512, g0 + w - 512)]
                                else:
                                    pieces = [(g0, w)]
                                off = 0
                                for pg0, pw in pieces:
                                    stop = (kc == 4 and pg0 + pw == 512) or \
                                           (kc == 7 and pg0 + pw == G)
                                    nc.tensor.matmul(
                                        pv[:, pg0: pg0 + pw],
                                        vh_panel[:, kc, h, :],
                                        p_sb[:, 256 * j + off: 256 * j + off + pw],
                                        start=False, stop=stop,
                                        skip_group_check=True,
                                    )
                                    off += pw

                        # ---- normalize + evacuate ----
                        # (DVE is lane-aligned: odd heads go through a temp
                        # tile + partition-shifting SBUF->SBUF DMA)
                        zsb = small.tile([1, G], F32, tag="zsb")
                        nc.vector.tensor_copy(zsb[:], pv[DH: DH + 1, :])
                        rz = small.tile([1, G], F32, tag="rz")
                        nc.vector.reciprocal_approx_fast(rz[:], zsb[:])
                        rzb = bcast.tile([64, G], F32, tag="rzb")
                        nc.gpsimd.partition_broadcast(rzb[:], rz[:])
                        if hh == 0:
                            nc.vector.tensor_tensor(
                                attnout[hrows, hp, 1:NT], pv[0:DH, :], rzb[:],
                                ALU.mult)
                        else:
                            tmp = bcast.tile([64, G], BF16, tag="tmp1")
                            nc.vector.tensor_tensor(
                                tmp[:], pv[0:DH, :], rzb[:], ALU.mult)
                            nc.sync.dma_start(attnout[hrows, hp, 1:NT], tmp[:])

                # ---- BOS-query epilogue (needs all heads' zq) ----
                rzq = small.tile([1, HEADS], F32, tag="rzq")
                nc.vector.reciprocal_approx_fast(rzq[:], zq_sb[:])
                rzqb = small.tile([P, HEADS], F32, tag="rzqb")
                nc.gpsimd.partition_broadcast(rzqb[:], rzq[:])
                for h in range(HEADS):
                    hp, hh = divmod(h, 2)
                    hrows = slice(64 * hh, 64 * (hh + 1))
                    pq = pq_all[:, h, :]
                    nc.vector.tensor_tensor(
                        pq, pq,
                        rzqb[:, h: h + 1].to_broadcast([P, 9]), ALU.mult)
                    pvq = psA.tile([P, 512], F32, tag="ps")
                    nc.tensor.matmul(pvq[0:DH + 1, 0:1], vbos_panel[:, h, :],
                                     pq[0:1, 0:1], start=True, stop=False,
                                     skip_group_check=True)
                    for c in range(8):
                        nc.tensor.matmul(
                            pvq[0:DH + 1, 0:1], vh_panel[:, c, h, :],
                            pq[:, 1 + c: 2 + c], start=False, stop=(c == 7),
                            skip_group_check=True,
                        )
                    if hh == 0:
                        nc.vector.tensor_copy(attnout[hrows, hp, 0:1],
                                              pvq[0:DH, 0:1])
                    else:
                        tmpb = small.tile([64, 1], BF16, tag="tmpb")
                        nc.vector.tensor_copy(tmpb[:], pvq[0:DH, 0:1])
                        nc.sync.dma_start(attnout[hrows, hp, 0:1], tmpb[:])

                # ---- output projection ----
                for mt in range(9):
                    t0 = 128 * mt
                    tw = 128 if mt < 8 else 1
                    for half in range(2):
                        acc = psA.tile([P, 512], F32, tag="ps")
                        for ct in range(8):
                            nc.tensor.matmul(
                                acc[0:tw, :],
                                attnout[:, ct, t0: t0 + tw],
                                wo_sb[:, ct, 512 * half: 512 * (half + 1)],
                                start=(ct == 0), stop=(ct == 7),
                            )
                        ost = bcast.tile([P, 512], F32, tag="ost")
                        nc.vector.tensor_copy(ost[0:tw, :], acc[0:tw, :])
                        nc.sync.dma_start(
                            out[b, t0: t0 + tw, 512 * half: 512 * (half + 1)],
                            ost[0:tw, :],
                        )

    nc.compile()
    return nc


_NC = None


def _get_nc():
    global _NC
    if _NC is None:
        _NC = build_nc()
    return _NC


def kernel(q, k, v, Wq, Wk, Wv, Wo, bo):
    bf16 = ml_dtypes.bfloat16
    qT = np.ascontiguousarray(np.asarray(q, np.float32).transpose(0, 2, 1)).astype(bf16)
    kT = np.ascontiguousarray(np.asarray(k, np.float32).transpose(0, 2, 1)).astype(bf16)
    vT = np.ascontiguousarray(np.asarray(v, np.float32).transpose(0, 2, 1)).astype(bf16)
    wq16 = np.asarray(Wq, np.float32).astype(bf16)
    wk16 = np.asarray(Wk, np.float32).astype(bf16)
    wv16 = np.asarray(Wv, np.float32).astype(bf16)
    wo16 = np.asarray(Wo, np.float32).astype(bf16)

    nc = _get_nc()
    in_maps = []
    for c in range(NCORES):
        sl = slice(BPC * c, BPC * (c + 1))
        in_maps.append({
            "qt": np.ascontiguousarray(qT[sl]),
            "kt": np.ascontiguousarray(kT[sl]),
            "vt": np.ascontiguousarray(vT[sl]),
            "wq": wq16, "wk": wk16, "wv": wv16, "wo": wo16,
        })
    res = run_bass_kernel_spmd(nc, in_maps, core_ids=list(range(NCORES)))
    out = np.concatenate([r["out"] for r in res.results], axis=0)
    out = out + np.asarray(bo, np.float32)[None, None, :]
    return out.astype(np.float32)


if __name__ == "__main__":
    rng = np.random.default_rng(0)
    ins = {
        "q": rng.standard_normal((B, NT, DIM), np.float32),
        "k": rng.standard_normal((B, NT, DIM), np.float32),
        "v": rng.standard_normal((B, NT, DIM), np.float32),
        "Wq": rng.standard_normal((DIM, INNER), np.float32) * DIM ** -0.5,
        "Wk": rng.standard_normal((DIM, INNER), np.float32) * DIM ** -0.5,
        "Wv": rng.standard_normal((DIM, INNER), np.float32) * DIM ** -0.5,
        "Wo": rng.standard_normal((INNER, DIM), np.float32) * INNER ** -0.5,
        "bo": np.zeros((DIM,), np.float32),
    }
    o = kernel(**ins)
    print(o.shape, o.dtype, np.abs(o).max())


# revision 16
# speedup vs baseline: 1.2759x; 1.0014x over previous
"""NearbyAttention Trainium2 kernel.

Full-input contract: kernel(**inputs) takes the unsharded numpy inputs of
nn_NearbyAttention (q,k,v: [16,1025,1024] f32; Wq/Wk/Wv/Wo: [1024,1024] f32;
bo: [1024] f32) and returns the full [16,1025,1024] f32 output.

Strategy: 8-way data parallel over the batch dim (2 batches per NeuronCore),
weights replicated, no collectives. Host pre-transposes activations to
[dim, tokens] and casts to bf16. On device, per batch:
  - projections on the PE array (bf16, fp32 PSUM accumulation)
  - sparse "nearby" attention exploiting the 5x5 locality mask:
    key chunks of 128 (4 patch rows) x 256-query windows in S^T layout,
    exp on the scalar engine, multiplicative mask, PV accumulation with a
    ones-column producing the softmax denominator for free
  - output projection back to [tokens, 1024] fp32
"""

import numpy as np
import ml_dtypes

import concourse.bass as bass
import concourse.mybir as mybir
import concourse.tile as tile
from concourse import bacc
from concourse.bass_utils import run_bass_kernel_spmd

BF16 = mybir.dt.bfloat16
F32 = mybir.dt.float32
AF = mybir.ActivationFunctionType
ALU = mybir.AluOpType

B = 16              # full batch
BPC = 2             # batches per core
NCORES = 8
NT = 1025           # tokens (BOS + 32*32 grid)
G = 1024            # grid tokens
DIM = 1024
HEADS = 16
DH = 64
INNER = HEADS * DH  # 1024
P = 128
SCALE = DH ** -0.5  # 0.125
NEG = -30.0         # mask bias for the handful of memset'd cells

KC = 8              # key chunks of 128 grid tokens (4 patch rows each)


def _mask_pattern() -> np.ndarray:
    """[128 key-local, 256 query-local] 0/1 bf16 mask in S^T orientation.

    Key chunk rows kr=0..3 (absolute 4kc+kr); query window rows
    rho=0..7 (absolute 4kc-2+rho). Unmasked iff |kr+2-rho|<=2 and
    |kcol-qcol|<=2.
    """
    kr = np.arange(128)[:, None] // 32
    kcol = np.arange(128)[:, None] % 32
    qr = np.arange(256)[None, :] // 32
    qcol = np.arange(256)[None, :] % 32
    m = (np.abs(kr + 2 - qr) <= 2) & (np.abs(kcol - qcol) <= 2)
    return m.astype(ml_dtypes.bfloat16)


def _qwin(kc: int) -> tuple[int, int, int]:
    """Grid-query window for key chunk kc: (grid_start, width, mask_col_off)."""
    if kc == 0:
        return 0, 192, 64
    if kc == KC - 1:
        return 128 * kc - 64, 192, 0
    return 128 * kc - 64, 256, 0


def build_nc():
    nc = bacc.Bacc("TRN2", target_bir_lowering=False, debug=False,
                   num_devices=NCORES)

    qt = nc.dram_tensor("qt", [BPC, DIM, NT], BF16, kind="ExternalInput")
    kt = nc.dram_tensor("kt", [BPC, DIM, NT], BF16, kind="ExternalInput")
    vt = nc.dram_tensor("vt", [BPC, DIM, NT], BF16, kind="ExternalInput")
    wq = nc.dram_tensor("wq", [DIM, INNER], BF16, kind="ExternalInput")
    wk = nc.dram_tensor("wk", [DIM, INNER], BF16, kind="ExternalInput")
    wv = nc.dram_tensor("wv", [DIM, INNER], BF16, kind="ExternalInput")
    wo = nc.dram_tensor("wo", [INNER, DIM], BF16, kind="ExternalInput")
    out = nc.dram_tensor("out", [BPC, NT, DIM], F32, kind="ExternalOutput")

    mask_np = _mask_pattern()  # [128, 256]
    mask2_np = np.concatenate([mask_np, mask_np], axis=1)  # [128, 512]
    mask_dram = nc.inline_tensor(mask2_np, name="mask2")

    with tile.TileContext(nc) as tc:
        with (
            tc.tile_pool(name="singles", bufs=1) as singles,
            tc.tile_pool(name="perbatch", bufs=1) as perbatch,
            tc.tile_pool(name="hppool", bufs=3) as hppool,
            tc.tile_pool(name="ppool", bufs=3) as ppool,
            tc.tile_pool(name="small", bufs=2) as small,
            tc.tile_pool(name="bcast", bufs=2) as bcast,
            tc.tile_pool(name="psA", bufs=4, space="PSUM") as psA,
            tc.tile_pool(name="psPV", bufs=2, space="PSUM") as psPV,
        ):
            # ---- persistent weights/constants ----
            wq_sb = singles.tile([P, 8, INNER], BF16, tag="wq")
            wk_sb = singles.tile([P, 8, INNER], BF16, tag="wk")
            wv_sb = singles.tile([P, 8, INNER], BF16, tag="wv")
            wo_sb = singles.tile([P, 8, DIM], BF16, tag="wo")
            mask2_sb = singles.tile([P, 512], BF16, tag="mask2")
            ones_sb = singles.tile([P, 1], BF16, tag="ones")

            nc.sync.dma_start(wq_sb[:], wq.ap().rearrange("(c p) n -> p c n", p=P))
            nc.sync.dma_start(wk_sb[:], wk.ap().rearrange("(c p) n -> p c n", p=P))
            nc.sync.dma_start(wv_sb[:], wv.ap().rearrange("(c p) n -> p c n", p=P))
            nc.sync.dma_start(wo_sb[:], wo.ap().rearrange("(c p) n -> p c n", p=P))
            nc.sync.dma_start(mask2_sb[:], mask_dram[:])
            nc.vector.memset(ones_sb[:], 1.0)

            for b in range(BPC):
                # ---- load transposed activations ----
                qT = perbatch.tile([P, 8, NT], BF16, tag="qT")
                kT = perbatch.tile([P, 8, NT], BF16, tag="kT")
                vT = perbatch.tile([P, 8, NT], BF16, tag="vT")
                nc.sync.dma_start(qT[:], qt[b].rearrange("(c p) n -> p c n", p=P))
                nc.sync.dma_start(kT[:], kt[b].rearrange("(c p) n -> p c n", p=P))
                nc.sync.dma_start(vT[:], vt[b].rearrange("(c p) n -> p c n", p=P))

                # ---- v projection into head panels ----
                # vh_panel[g, kc, h, 0:64] = (v @ Wv)[token g+1, h*64:...]
                # col 64 = ones (softmax denominator trick)
                vh_panel = perbatch.tile([P, KC, HEADS, DH + 1], BF16, tag="vhp")
                vbos_panel = perbatch.tile([1, HEADS, DH + 1], BF16, tag="vbos")
                nc.vector.memset(vh_panel[:, :, :, DH], 1.0)
                nc.vector.memset(vbos_panel[:, :, DH], 1.0)

                for mt in range(KC):  # grid token chunks
                    for half in range(2):
                        acc = psA.tile([P, 512], F32, tag="ps")
                        for ktile in range(8):
                            nc.tensor.matmul(
                                acc[:],
                                vT[:, ktile, 1 + 128 * mt: 1 + 128 * (mt + 1)],
                                wv_sb[:, ktile, 512 * half: 512 * (half + 1)],
                                start=(ktile == 0), stop=(ktile == 7),
                            )
                        nc.vector.tensor_copy(
                            vh_panel[:, mt, 8 * half: 8 * (half + 1), 0:DH],
                            acc.rearrange("p (h d) -> p h d", d=DH),
                        )
                # BOS token of v
                for half in range(2):
                    acc = psA.tile([P, 512], F32, tag="ps")
                    for ktile in range(8):
                        nc.tensor.matmul(
                            acc[0:1, :], vT[:, ktile, 0:1],
                            wv_sb[:, ktile, 512 * half: 512 * (half + 1)],
                            start=(ktile == 0), stop=(ktile == 7),
                        )
                    nc.vector.tensor_copy(
                        vbos_panel[:, 8 * half: 8 * (half + 1), 0:DH],
                        acc[0:1].rearrange("p (h d) -> p h d", d=DH),
                    )

                attnout = perbatch.tile([P, 8, NT], BF16, tag="attnout")
                zq_sb = small.tile([1, HEADS], F32, tag="zq")
                pq_all = perbatch.tile([P, HEADS, 9], BF16, tag="pqall")

                for hp in range(8):
                    # ---- q/k head-pair projections (inner chunk = hp) ----
                    qhT = hppool.tile([P, NT], BF16, tag="qhT")
                    khT = hppool.tile([P, NT], BF16, tag="khT")
                    for dst, w_sb, src, scl in (
                        (qhT, wq_sb, qT, SCALE), (khT, wk_sb, kT, 1.0),
                    ):
                        for nt0, ntw in ((0, 512), (512, 512), (1024, 1)):
                            acc = psA.tile([P, 512], F32, tag="ps")
                            for ktile in range(8):
                                nc.tensor.matmul(
                                    acc[:, 0:ntw],
                                    w_sb[:, ktile, 128 * hp: 128 * (hp + 1)],
                                    src[:, ktile, nt0: nt0 + ntw],
                                    start=(ktile == 0), stop=(ktile == 7),
                                )
                            if scl == 1.0:
                                nc.vector.tensor_copy(
                                    dst[:, nt0: nt0 + ntw], acc[:, 0:ntw])
                            else:
                                nc.scalar.activation(
                                    dst[:, nt0: nt0 + ntw], acc[:, 0:ntw],
                                    AF.Copy, scale=scl)

                    HH = (slice(0, 64), slice(64, 128))

                    # ---- BOS-as-key scores for all grid queries ----
                    pbos = [small.tile([1, G], BF16, tag=f"pbos{hh}", name=f"pbos{hh}")
                            for hh in range(2)]
                    for c0 in (0, 512):
                        sb2 = [psA.tile([P, 512], F32, tag="ps", name=f"sb2_{_i}")
                               for _i in range(2)]
                        for hh in range(2):
                            nc.tensor.matmul(
                                sb2[hh][0:1, :], khT[HH[hh], 0:1],
                                qhT[HH[hh], 1 + c0: 1 + c0 + 512],
                                start=True, stop=True,
                            )
                        for hh in range(2):
                            nc.scalar.activation(
                                pbos[hh][:, c0: c0 + 512], sb2[hh][0:1, :],
                                AF.Exp)

                    # ---- BOS-as-query scores over everything ----
                    sq2 = [psA.tile([P, 512], F32, tag="ps", name=f"sq2_{_i}") for _i in range(2)]
                    for hh in range(2):
                        nc.vector.memset(sq2[hh][:, 0:1], NEG)
                    for hh in range(2):
                        nc.tensor.matmul(sq2[hh][0:1, 0:1], khT[HH[hh], 0:1],
                                         qhT[HH[hh], 0:1], start=True, stop=True)
                        for c in range(8):
                            nc.tensor.matmul(
                                sq2[hh][:, 1 + c: 2 + c],
                                khT[HH[hh], 1 + 128 * c: 1 + 128 * (c + 1)],
                                qhT[HH[hh], 0:1], start=True, stop=True,
                            )
                    zrow = psA.tile([P, 512], F32, tag="ps")
                    for hh in range(2):
                        h = 2 * hp + hh
                        pq = pq_all[:, h, :]
                        nc.scalar.activation(pq, sq2[hh][:, 0:9], AF.Exp)
                        nc.tensor.matmul(zrow[0:1, 9 * hh: 9 * hh + 9],
                                         ones_sb[:], pq,
                                         start=True, stop=True)
                        nc.vector.tensor_reduce(
                            zq_sb[:, h: h + 1], zrow[0:1, 9 * hh: 9 * hh + 9],
                            axis=mybir.AxisListType.X, op=ALU.add)

                    # ---- main nearby attention, both heads interleaved ----
                    pv2 = [psPV.tile([DH + 1, G], F32, tag="pv", name=f"pv{_i}")
                           for _i in range(2)]
                    for hh in range(2):
                        h = 2 * hp + hh
                        for c0 in (0, 512):
                            nc.tensor.matmul(
                                pv2[hh][:, c0: c0 + 512], vbos_panel[:, h, :],
                                pbos[hh][:, c0: c0 + 512],
                                start=True, stop=False, skip_group_check=True)

                    for kcp in range(4):  # pairs of key chunks
                        kca, kcb = 2 * kcp, 2 * kcp + 1
                        s2 = [psA.tile([P, 512], F32, tag="ps", name=f"s2_{_i}")
                              for _i in range(2)]
                        p2 = [ppool.tile([P, 512], BF16, tag="p", name=f"p2_{_i}")
                              for _i in range(2)]
                        # all four QK matmuls adjacent: h0/h1 use disjoint
                        # PE row groups and run concurrently
                        for j, kc in enumerate((kca, kcb)):
                            g0, w, _ = _qwin(kc)
                            for hh in range(2):
                                nc.tensor.matmul(
                                    s2[hh][:, 256 * j: 256 * j + w],
                                    khT[HH[hh], 1 + 128 * kc: 1 + 128 * (kc + 1)],
                                    qhT[HH[hh], 1 + g0: 1 + g0 + w],
                                    start=True, stop=True,
                                )
                            if w < 256:
                                for hh in range(2):
                                    nc.vector.memset(
                                        s2[hh][:, 256 * j + w: 256 * (j + 1)],
                                        0.0)
                        ma = _qwin(kca)[2]
                        mb = _qwin(kcb)[2]
                        for hh in range(2):
                            nc.scalar.activation(p2[hh][:], s2[hh][:], AF.Exp)
                            if ma == 0 and mb == 0:
                                nc.vector.tensor_tensor(
                                    p2[hh][:], p2[hh][:], mask2_sb[:], ALU.mult)
                            else:
                                wa = _qwin(kca)[1]
                                wb = _qwin(kcb)[1]
                                nc.vector.tensor_tensor(
                                    p2[hh][:, 0:wa], p2[hh][:, 0:wa],
                                    mask2_sb[:, ma: ma + wa], ALU.mult)
                                nc.vector.tensor_tensor(
                                    p2[hh][:, 256: 256 + wb],
                                    p2[hh][:, 256: 256 + wb],
                                    mask2_sb[:, mb: mb + wb], ALU.mult)
                        # PV accumulation (split at the PSUM bank boundary)
                        for hh in range(2):
                            h = 2 * hp + hh
                            for j, kc in enumerate((kca, kcb)):
                                g0, w, _ = _qwin(kc)
                                if g0 < 512 and g0 + w > 512:
                                    pieces = [(g0, 512 - g0), (512, g0 + w - 512)]
                                else:
                                    pieces = [(g0, w)]
                                off = 0
                                for pg0, pw in pieces:
                                    stop = (kc == 4 and pg0 + pw == 512) or \
                                           (kc == 7 and pg0 + pw == G)
                                    nc.tensor.matmul(
                                        pv2[hh][:, pg0: pg0 + pw],
                                        vh_panel[:, kc, h, :],
                                        p2[hh][:, 256 * j + off: 256 * j + off + pw],
                                        start=False, stop=stop,
                                        skip_group_check=True,
                                    )
                                    off += pw

                    # ---- normalize + evacuate ----
                    # (DVE is lane-aligned: odd heads go through a temp
                    # tile + partition-shifting SBUF->SBUF DMA)
                    for hh in range(2):
                        pv = pv2[hh]
                        zsb = small.tile([1, G], F32, tag="zsb")
                        nc.vector.tensor_copy(zsb[:], pv[DH: DH + 1, :])
                        rz = small.tile([1, G], F32, tag="rz")
                        nc.vector.reciprocal_approx_fast(rz[:], zsb[:])
                        rzb = bcast.tile([64, G], F32, tag="rzb")
                        nc.gpsimd.partition_broadcast(rzb[:], rz[:])
                        if hh == 0:
                            nc.vector.tensor_tensor(
                                attnout[HH[0], hp, 1:NT], pv[0:DH, :], rzb[:],
                                ALU.mult)
                        else:
                            tmp = bcast.tile([64, G], BF16, tag="tmp1")
                            nc.vector.tensor_tensor(
                                tmp[:], pv[0:DH, :], rzb[:], ALU.mult)
                            nc.sync.dma_start(attnout[HH[1], hp, 1:NT], tmp[:])

                # ---- BOS-query epilogue (needs all heads' zq) ----
                rzq = small.tile([1, HEADS], F32, tag="rzq")
                nc.vector.reciprocal_approx_fast(rzq[:], zq_sb[:])
                rzqb = small.tile([P, HEADS], F32, tag="rzqb")
                nc.gpsimd.partition_broadcast(rzqb[:], rzq[:])
                for h in range(HEADS):
                    hp, hh = divmod(h, 2)
                    hrows = slice(64 * hh, 64 * (hh + 1))
                    pq = pq_all[:, h, :]
                    nc.vector.tensor_tensor(
                        pq, pq,
                        rzqb[:, h: h + 1].to_broadcast([P, 9]), ALU.mult)
                    pvq = psA.tile([P, 512], F32, tag="ps")
                    nc.tensor.matmul(pvq[0:DH + 1, 0:1], vbos_panel[:, h, :],
                                     pq[0:1, 0:1], start=True, stop=False,
                                     skip_group_check=True)
                    for c in range(8):
                        nc.tensor.matmul(
                            pvq[0:DH + 1, 0:1], vh_panel[:, c, h, :],
                            pq[:, 1 + c: 2 + c], start=False, stop=(c == 7),
                            skip_group_check=True,
                        )
                    if hh == 0:
                        nc.vector.tensor_copy(attnout[hrows, hp, 0:1],
                                              pvq[0:DH, 0:1])
                    else:
                        tmpb = small.tile([64, 1], BF16, tag="tmpb")
                        nc.vector.tensor_copy(tmpb[:], pvq[0:DH, 0:1])
                        nc.sync.dma_start(attnout[hrows, hp, 0:1], tmpb[:])

                # ---- output projection ----
                for mt in range(9):
                    t0 = 128 * mt
                    tw = 128 if mt < 8 else 1
                    for half in range(2):
                        acc = psA.tile([P, 512], F32, tag="ps")
                        for ct in range(8):
                            nc.tensor.matmul(
                                acc[0:tw, :],
                                attnout[:, ct, t0: t0 + tw],
                                wo_sb[:, ct, 512 * half: 512 * (half + 1)],
                                start=(ct == 0), stop=(ct == 7),
                            )
                        ost = bcast.tile([P, 512], F32, tag="ost")
                        nc.vector.tensor_copy(ost[0:tw, :], acc[0:tw, :])
                        nc.sync.dma_start(
                            out[b, t0: t0 + tw, 512 * half: 512 * (half + 1)],
                            ost[0:tw, :],
                        )

    nc.compile()
    return nc


_NC = None


def _get_nc():
    global _NC
    if _NC is None:
        _NC = build_nc()
    return _NC


def kernel(q, k, v, Wq, Wk, Wv, Wo, bo):
    bf16 = ml_dtypes.bfloat16
    qT = np.ascontiguousarray(np.asarray(q, np.float32).transpose(0, 2, 1)).astype(bf16)
    kT = np.ascontiguousarray(np.asarray(k, np.float32).transpose(0, 2, 1)).astype(bf16)
    vT = np.ascontiguousarray(np.asarray(v, np.float32).transpose(0, 2, 1)).astype(bf16)
    wq16 = np.asarray(Wq, np.float32).astype(bf16)
    wk16 = np.asarray(Wk, np.float32).astype(bf16)
    wv16 = np.asarray(Wv, np.float32).astype(bf16)
    wo16 = np.asarray(Wo, np.float32).astype(bf16)

    nc = _get_nc()
    in_maps = []
    for c in range(NCORES):
        sl = slice(BPC * c, BPC * (c + 1))
        in_maps.append({
            "qt": np.ascontiguousarray(qT[sl]),
            "kt": np.ascontiguousarray(kT[sl]),
            "vt": np.ascontiguousarray(vT[sl]),
            "wq": wq16, "wk": wk16, "wv": wv16, "wo": wo16,
        })
    res = run_bass_kernel_spmd(nc, in_maps, core_ids=list(range(NCORES)))
    out = np.concatenate([r["out"] for r in res.results], axis=0)
    out = out + np.asarray(bo, np.float32)[None, None, :]
    return out.astype(np.float32)


if __name__ == "__main__":
    rng = np.random.default_rng(0)
    ins = {
        "q": rng.standard_normal((B, NT, DIM), np.float32),
        "k": rng.standard_normal((B, NT, DIM), np.float32),
        "v": rng.standard_normal((B, NT, DIM), np.float32),
        "Wq": rng.standard_normal((DIM, INNER), np.float32) * DIM ** -0.5,
        "Wk": rng.standard_normal((DIM, INNER), np.float32) * DIM ** -0.5,
        "Wv": rng.standard_normal((DIM, INNER), np.float32) * DIM ** -0.5,
        "Wo": rng.standard_normal((INNER, DIM), np.float32) * INNER ** -0.5,
        "bo": np.zeros((DIM,), np.float32),
    }
    o = kernel(**ins)
    print(o.shape, o.dtype, np.abs(o).max())
